# revision 42
# baseline (speedup 1.0000x reference)
"""DGCNN (nn_DGCNN_11106785427638) Trainium2 Bass kernel.

Pure data-parallel: B=8 samples sharded 1-per-core across 8 NeuronCores.
Per core (N=2048 points, k=20, f32 throughout):

  Per EdgeConv block b=1..4 (C=3,64,64,64):
    - knn: dist tile (128 rows, 2048) via PE matmul with the -|x_m|^2 term
      folded in as an extra contraction row: s[n,m] = 2<x_n,x_m> - |x_m|^2
      (rank-equivalent to the reference's pairwise -dist^2).
    - exact sorted top-20 via DVE max8/max_index/match_replace (3 rounds).
    - neighbor gather runs on GPSIMD ap_gather over A_b = Wa_b @ x_b
      (the 1x1 conv is linear, so conv(gather(x)) == gather(A)); the center
      term Bv_b = (Wb_b - Wa_b) @ x_b is added per-point afterwards.
    - x_{b+1} = prelu(maxpool_j(gather(A)) * s + (Bv*s + bias)) using the
      monotonicity of the (positive-scale) BN + leaky relu.
  Phase 2: y_b = prelu((A_b[idx] )*s + BvS_b) at full (64, N, 20), conv5 as
  two K=128 pair-stacked matmuls, maxpool_j, conv6, global max/mean pool,
  3-layer MLP head. Weights/BN folds are host-preprocessed.

Flat gather order within a 128-row tile: i = a*320 + j*16 + p with local
row n = 16a + p, neighbor rank j. The idx "wrap" for ap_gather is one
SBUF->DRAM bounce in (p, a, j) layout plus stride-0 broadcast reads that
replicate it per 16-partition core group (1 DMA for idxw, 2 for idxh),
splits phase-1 across all 8 Q7 cores by row-halves, and keeps pooled
outputs n-contiguous.

Scheduling: S2 = [2x; -|x|^2] for block b+1 is built incrementally per
128-col chunk as S1[b] tiles complete (block 1's S2 and [x;1] come
host-side), each block's first two dist tiles are emitted ahead of the
aa/bb prestep, and conv6 + global pools run chunked inside the phase-2
loop - together hiding most block-transition/startup/tail bubbles. The
exact top-20 (3x max8/max_index + 2x match_replace = 8 DVE passes/tile)
is the DVE-bound floor: ~1.34 ms busy of ~1.44 ms total.
"""
import sys

sys.path.insert(0, "/opt/trn_rl_repo")
import numpy as np
import concourse.bacc as bacc
import concourse.tile as tile
from concourse import mybir
from concourse.bass_utils import run_bass_kernel_spmd
import contextlib
from contextlib import ExitStack

FP = mybir.dt.float32
U32 = mybir.dt.uint32
U16 = mybir.dt.uint16
I16 = mybir.dt.int16
AF = mybir.ActivationFunctionType
ALU = mybir.AluOpType
AX = mybir.AxisListType

B, N, K, CLS = 8, 2048, 20, 40
NT = N // 128          # 16 row tiles
NEG = -3.0e38
EPS = 1e-5
CS = [3, 64, 64, 64]   # per-block input channels

_CACHE = {}


def _build():
    nc = bacc.Bacc("TRN2", target_bir_lowering=False, debug=False)

    d = {}
    def din(name, shape, dt=FP):
        d[name] = nc.dram_tensor(name, list(shape), dt, kind="ExternalInput").ap()
        return d[name]

    din("x3", (4, N))
    din("s2one", (4, N))
    din("onesrow", (1, N))
    for b in range(1, 5):
        C = CS[b - 1]
        din(f"waaT{b}", (C, 128))
        din(f"wddT{b}", (C, 128))
        din(f"scdup{b}", (128, 1))
        din(f"bidup{b}", (128, 1))
    din("ident", (128, 128))
    din("w5T12", (128, 64)); din("w5T34", (128, 64))
    din("sc5", (64, 1)); din("bi5", (64, 1))
    din("w6Ta", (64, 128)); din("w6Tb", (64, 128))
    din("sc6a", (128, 1)); din("bi6a", (128, 1))
    din("sc6b", (128, 1)); din("bi6b", (128, 1))
    din("l1T", (128, 1024))
    din("sc1a", (128, 1)); din("bi1a", (128, 1))
    din("sc1b", (128, 1)); din("bi1b", (128, 1))
    din("l2T", (128, 128)); din("sc2", (64, 1)); din("bi2", (64, 1))
    din("l3T", (64, CLS)); din("bi3", (CLS, 1))
    out_d = nc.dram_tensor("out", [CLS, 1], FP, kind="ExternalOutput").ap()

    bounce_m = {}
    for b in range(1, 5):
        for t in range(NT):
            bounce_m[(b, t)] = nc.dram_tensor(f"bncm{b}_{t}", [16, 160], U16)

    with tile.TileContext(nc) as tc, ExitStack() as ctx:
        cp = ctx.enter_context(tc.tile_pool(name="consts", bufs=1))
        sb = {}
        # block-1-critical inputs first so the first dist tile starts ASAP;
        # remaining consts (phase-2/tail weights) stream in behind them.
        early = ["x3", "s2one", "onesrow", "waaT1", "wddT1", "scdup1", "bidup1"]
        for name in early + [n for n in d if n not in early]:
            ap = d[name]
            t_ = cp.tile(list(ap.shape), ap.dtype, name=f"c_{name}", tag=f"c_{name}")
            nc.sync.dma_start(t_[:], ap[:])
            sb[name] = t_
        negones = cp.tile([64, 1], FP)
        nc.vector.memset(negones[:], -1.0)
        # dummy activation: pulls the ACT function-table load into the
        # initial const-DMA wait instead of the first dist-copy
        actwarm = cp.tile([64, 1], FP)
        nc.scalar.activation(actwarm[:], negones[:], AF.Prelu, alpha=0.2)

        persist = ctx.enter_context(tc.tile_pool(name="persist", bufs=1))
        S1 = {i: persist.tile([65, N], FP, name=f"S1_{i}", tag=f"S1_{i}") for i in (1, 2, 3)}
        pairA = {p: persist.tile([128, N], FP, name=f"pairA{p}", tag=f"pairA{p}") for p in (12, 34)}
        BvSpair = {p: persist.tile([128, N], FP, name=f"BvSpair{p}", tag=f"BvSpair{p}") for p in (12, 34)}
        idxw = {p: persist.tile([128, NT * 160], U16, name=f"idxw{p}", tag=f"idxw{p}") for p in (12, 34)}
        hmax = persist.tile([64, N], FP)
        g12pre = {0: persist.tile([128, 2560], FP, name="g12pre0", tag="g12pre0")}

        # phase-2 front stage (gather -> +BvS -> prelu); also used to pre-run
        # pair 12 of tile 0 during block 4 (its inputs are ready after block 2)
        def front_one(pair, t, G):
            nc.gpsimd.ap_gather(G[:], pairA[pair][:],
                                idxw[pair][:, t * 160:(t + 1) * 160].bitcast(I16),
                                channels=128, num_elems=N, d=1, num_idxs=2560)
            for a in range(8):
                gv = G[:, a * 320:(a + 1) * 320].rearrange(
                    "c (j p) -> c j p", j=20, p=16)
                bvv = BvSpair[pair][:, t * 128 + a * 16:t * 128 + (a + 1) * 16] \
                    .rearrange("c (u2 p) -> c u2 p", u2=1) \
                    .broadcast_to([128, 20, 16])
                nc.vector.scalar_tensor_tensor(gv, gv, 1.0, bvv,
                                               op0=ALU.mult, op1=ALU.add)
            for g in range(4):
                nc.scalar.activation(G[:, g * 640:(g + 1) * 640],
                                     G[:, g * 640:(g + 1) * 640],
                                     AF.Prelu, alpha=0.2)
            return G

        # ---------------- phase 1: blocks ----------------
        with tc.tile_pool(name="s2p", bufs=2) as s2p, \
             tc.tile_pool(name="xsqc", bufs=2) as xsqcp, \
             tc.tile_pool(name="adup", bufs=1) as adupp, \
             tc.tile_pool(name="bvs", bufs=1) as bvsp, \
             tc.tile_pool(name="dist", bufs=3) as distp, \
             tc.tile_pool(name="gph1", bufs=2) as gph1p, \
             tc.tile_pool(name="small", bufs=4) as smallp, \
             tc.tile_pool(name="ps_pre", bufs=1, space="PSUM") as ps_pre, \
             tc.tile_pool(name="ps_dist", bufs=3, space="PSUM") as ps_dist:
            S2next = None
            pending_chunk = None
            for b in range(1, 5):
                C = CS[b - 1]
                pair = 12 if b <= 2 else 34
                half = slice(0, 64) if b % 2 == 1 else slice(64, 128)
                qbase = 0 if b % 2 == 1 else 4
                # block 1 reads [x3; 1] straight from the const tile; S2_1 is
                # host-computed ("s2one"); later blocks use the incrementally
                # built S1/S2next.
                xfull = sb["x3"] if b == 1 else S1[b - 1]
                x_b = xfull[0:C, :]
                S2 = sb["s2one"] if b == 1 else S2next
                if b <= 3:
                    S2next = s2p.tile([65, N], FP, tag="S2n")

                # first dist tile ahead of the aa/bb prestep: its inputs are
                # ready before x_b's last chunk, and PE/ACT queues are
                # in-order, so this shortens every block transition.
                def emit_dist(t, mid=None):
                    lhsT = xfull[:, t * 128:(t + 1) * 128]
                    dist = distp.tile([128, N], FP, name=f"dist_{b}_{t}", tag="dist")
                    for ch in range(2):
                        cs = slice(ch * 1024, (ch + 1) * 1024)
                        dps = ps_dist.tile([128, 1024], FP, name=f"dps_{b}_{t}_{ch}", tag="dch")
                        for u in range(2):
                            nc.tensor.matmul(dps[:, u * 512:(u + 1) * 512], lhsT,
                                             S2[0:C + 1, ch * 1024 + u * 512:ch * 1024 + (u + 1) * 512],
                                             start=True, stop=True)
                        nc.scalar.copy(dist[:, cs], dps[:])
                        if ch == 0 and mid is not None:
                            with tc.high_priority():
                                mid()
                    return dist
                # the previous block's tile-15 S2 chunk is emitted between
                # dist-t0's column halves: half 0 doesn't depend on it, so PE
                # starts early instead of head-of-line blocking on it.
                dist0 = emit_dist(0, mid=pending_chunk)
                pending_chunk = None
                dist1 = emit_dist(1)

                if b <= 3:
                    Adup = adupp.tile([128, N], FP, tag="adup")
                    BvSd = bvsp.tile([128, N], FP, tag="bvs")
                for hh in range(2):
                    hs = slice(hh * 1024, (hh + 1) * 1024)
                    aa = ps_pre.tile([128, 1024], FP, tag="pre")
                    for ch in range(2):
                        nc.tensor.matmul(aa[:, ch * 512:(ch + 1) * 512], sb[f"waaT{b}"][:],
                                         x_b[:, hh * 1024 + ch * 512:hh * 1024 + (ch + 1) * 512],
                                         start=True, stop=True)
                    nc.scalar.copy(pairA[pair][half, hs], aa[half, :])
                    if b <= 3:
                        nc.scalar.copy(Adup[:, hs], aa[:])

                    bb = ps_pre.tile([128, 1024], FP, tag="pre")
                    for ch in range(2):
                        nc.tensor.matmul(bb[:, ch * 512:(ch + 1) * 512], sb[f"wddT{b}"][:],
                                         x_b[:, hh * 1024 + ch * 512:hh * 1024 + (ch + 1) * 512],
                                         start=True, stop=True)
                    nc.scalar.activation(BvSpair[pair][half, hs], bb[half, :], AF.Identity,
                                         bias=sb[f"bidup{b}"][half, 0:1],
                                         scale=sb[f"scdup{b}"][half, 0:1])
                    if b <= 3:
                        nc.scalar.activation(BvSd[:, hs], bb[:], AF.Identity,
                                             bias=sb[f"bidup{b}"][:, 0:1],
                                             scale=sb[f"scdup{b}"][:, 0:1])
                if b <= 3:
                    nc.scalar.copy(S1[b][64:65, :], sb["onesrow"][:])
                if b == 4:
                    # pair-12 front work of phase-2 tile 0 (inputs ready once
                    # block 2 is done): fills the block-3/4 transition bubble
                    # on DVE/ACT/Pool
                    front_one(12, 0, g12pre[0])

                for t in range(NT):
                    dist = dist0 if t == 0 else (dist1 if t == 1 else emit_dist(t))

                    vals = smallp.tile([128, 24], FP, tag="vals")
                    idx16 = smallp.tile([128, 24], U16, tag="idx16")
                    for r in range(3):
                        nc.vector.max(vals[:, r * 8:(r + 1) * 8], dist[:])
                        nc.vector.max_index(idx16[:, r * 8:(r + 1) * 8],
                                            vals[:, r * 8:(r + 1) * 8], dist[:])
                        if r < 2:
                            nc.vector.match_replace(dist[:], vals[:, r * 8:(r + 1) * 8],
                                                    dist[:], NEG)

                    # the last tile's post-topk chain is the block-transition
                    # critical path: tell the scheduler to prefer it over
                    # competing ready work on the same engines
                    last = t == NT - 1
                    hp = tc.high_priority() if t >= NT - 2 else contextlib.nullcontext()
                    with hp:
                        # idx wrap via DRAM bounce in (p, a, j) layout; read
                        # back with a stride-0 4x quadrant broadcast
                        m_ap = bounce_m[(b, t)].ap()
                        nc.sync.dma_start(m_ap.rearrange("p (a j) -> a p j", a=8, j=20),
                                          idx16[:, 0:20])
                        nc.sync.dma_start(
                            idxw[pair][qbase * 16:(qbase + 4) * 16, t * 160:(t + 1) * 160],
                            m_ap.unsqueeze(0).broadcast_to([4, 16, 160]))

                        if b <= 3:
                            idxh = smallp.tile([128, 80], U16, tag="idxh")
                            nc.sync.dma_start(
                                idxh[0:64, :],
                                m_ap[:, 0:80].unsqueeze(0).broadcast_to([4, 16, 80]))
                            nc.sync.dma_start(
                                idxh[64:128, :],
                                m_ap[:, 80:160].unsqueeze(0).broadcast_to([4, 16, 80]))
                            G = gph1p.tile([128, 1280], FP, tag="g1")
                            nc.gpsimd.ap_gather(G[:], Adup[:], idxh[:].bitcast(I16),
                                                channels=128, num_elems=N, d=1, num_idxs=1280)
                            Rt = smallp.tile([128, 64], FP, tag="rt")
                            nc.vector.tensor_reduce(
                                Rt[:], G[:].rearrange("c (a j p) -> c a p j", a=4, j=20, p=16),
                                AX.X, ALU.max)
                            t1 = smallp.tile([128, 64], FP, tag="t1")
                            # t1 on Pool in steady state; on DVE for the last
                            # tile (DVE idles in the drain; skips a queue hop)
                            t1eng = nc.vector if last else nc.gpsimd
                            t1eng.tensor_tensor(
                                t1[0:64, :], Rt[0:64, :],
                                BvSd[0:64, t * 128:t * 128 + 64], ALU.add)
                            t1eng.tensor_tensor(
                                t1[64:128, :], Rt[64:128, :],
                                BvSd[64:128, t * 128 + 64:(t + 1) * 128], ALU.add)
                            t2 = smallp.tile([128, 64], FP, tag="t2")
                            nc.scalar.activation(t2[:], t1[:], AF.Prelu, alpha=0.2)
                            nc.scalar.copy(S1[b][0:64, t * 128:t * 128 + 64], t2[0:64, :])
                            nc.sync.dma_start(S1[b][0:64, t * 128 + 64:(t + 1) * 128],
                                              t2[64:128, :])

                            # incremental S2 build for block b+1 over these cols
                            def chunk_build(t=t, S1b=S1[b], S2n=S2next):
                                tcols = slice(t * 128, (t + 1) * 128)
                                xsqc = xsqcp.tile([64, 128], FP, tag="xsqc")
                                nc.scalar.activation(xsqc[:], S1b[0:64, tcols], AF.Square)
                                nxxc = ps_pre.tile([1, 128], FP, tag="pre")
                                nc.tensor.matmul(nxxc[:], negones[:], xsqc[:],
                                                 start=True, stop=True)
                                nc.scalar.mul(S2n[0:64, tcols], S1b[0:64, tcols], 2.0)
                                nc.scalar.copy(S2n[64:65, tcols], nxxc[:])
                            if not last:
                                chunk_build()
                            else:
                                pending_chunk = chunk_build

        # ---------------- phase 2: y + conv5 + pool ----------------
        # z = diag(s) @ G + I @ BvS (PE identity-adds; BvS broadcast over j
        # as a 0-stride rhs); y = prelu(z) with y12 on ACT, y34 on DVE via
        # max(0.2*v, v) to balance engine load.
        with tc.tile_pool(name="g2", bufs=3) as g2p, \
             tc.tile_pool(name="hsb", bufs=2) as hsbp, \
             tc.tile_pool(name="tail", bufs=1) as tp, \
             tc.tile_pool(name="ps_h", bufs=3, space="PSUM") as ps_h, \
             tc.tile_pool(name="ps_t6", bufs=2, space="PSUM") as ps_t6, \
             tc.tile_pool(name="ps_fc", bufs=2, space="PSUM") as ps_fc:
            # conv6 + global max/sum run chunked, interleaved into the
            # phase-2 loop as hmax columns complete
            gmall = tp.tile([128, 8], FP, tag="gmall")
            gsall = tp.tile([128, 8], FP, tag="gsall")
            W6 = (("w6Ta", "sc6a", "bi6a"), ("w6Tb", "sc6b", "bi6b"))

            def conv6_chunk(ch):
                cs = slice(ch * 512, (ch + 1) * 512)
                for wi, (wname, scn, bin_) in enumerate(W6):
                    z6 = ps_t6.tile([128, 512], FP, name=f"z6_{ch}_{wi}", tag="z6")
                    nc.tensor.matmul(z6[:], sb[wname][:], hmax[:, cs],
                                     start=True, stop=True)
                    h6 = tp.tile([128, 512], FP, name=f"h6_{ch}_{wi}",
                                 tag="h6c", bufs=2)
                    nc.scalar.activation(h6[:], z6[:], AF.Prelu,
                                         bias=sb[bin_][:, 0:1],
                                         scale=sb[scn][:, 0:1], alpha=0.2)
                    col = slice(wi * 4 + ch, wi * 4 + ch + 1)
                    nc.vector.tensor_reduce(gmall[:, col], h6[:], AX.X, ALU.max)
                    nc.vector.tensor_reduce(gsall[:, col], h6[:], AX.X, ALU.add)
            def phase2_front(t):
                ys = {}
                for pair in (12, 34):
                    if pair == 12 and t == 0:
                        ys[12] = g12pre[0]  # pre-computed during block 4
                        continue
                    G = g2p.tile([128, 2560], FP, name=f"g{pair}_{t}", tag=f"g{pair}")
                    ys[pair] = front_one(pair, t, G)
                return ys

            def phase2_back(t, ys):
                h_sb = hsbp.tile([64, 2560], FP, name=f"hsb_{t}", tag="hsb")
                for ch in range(5):
                    cs = slice(ch * 512, (ch + 1) * 512)
                    hps = ps_h.tile([64, 512], FP, name=f"hps_{t}_{ch}", tag="h")
                    nc.tensor.matmul(hps[:], sb["w5T12"][:], ys[12][:, cs],
                                     start=True, stop=False)
                    nc.tensor.matmul(hps[:], sb["w5T34"][:], ys[34][:, cs],
                                     start=False, stop=True)
                    nc.scalar.activation(h_sb[:, cs], hps[:], AF.Prelu,
                                         bias=sb["bi5"][:, 0:1], scale=sb["sc5"][:, 0:1],
                                         alpha=0.2)
                nc.vector.tensor_reduce(
                    hmax[:, t * 128:(t + 1) * 128],
                    h_sb[:].rearrange("c (a j p) -> c a p j", a=8, j=20, p=16),
                    AX.X, ALU.max)

            ys_prev = None
            for t in range(NT + 1):
                ys_cur = phase2_front(t) if t < NT else None
                if ys_prev is not None:
                    phase2_back(t - 1, ys_prev)
                    if (t - 1) % 4 == 3:
                        ch = (t - 1) // 4
                        if ch == 3:
                            with tc.high_priority():
                                conv6_chunk(ch)
                        else:
                            conv6_chunk(ch)
                ys_prev = ys_cur

            # ---------------- tail: final pools + MLP ----------------
            tailhp = ctx.enter_context(tc.high_priority())
            gpieces = []
            for wi in range(2):
                gm = tp.tile([128, 1], FP, name=f"gm{wi}", tag=f"gm{wi}")
                nc.vector.tensor_reduce(gm[:], gmall[:, wi * 4:(wi + 1) * 4],
                                        AX.X, ALU.max)
                gs = tp.tile([128, 1], FP, name=f"gs{wi}", tag=f"gs{wi}")
                nc.vector.tensor_reduce(gs[:], gsall[:, wi * 4:(wi + 1) * 4],
                                        AX.X, ALU.add)
                gpieces.append((gm, gs))
            gchunks = [gpieces[0][0], gpieces[1][0], gpieces[0][1], gpieces[1][1]]

            z1sb = tp.tile([128, 2], FP, tag="z1")
            for o in range(2):
                z1 = ps_fc.tile([128, 1], FP, tag="fc")
                for k in range(4):
                    nc.tensor.matmul(z1[:], sb["l1T"][:, (k * 2 + o) * 128:(k * 2 + o + 1) * 128],
                                     gchunks[k][:], start=(k == 0), stop=(k == 3))
                nc.scalar.activation(z1sb[:, o:o + 1], z1[:], AF.Prelu,
                                     bias=sb["bi1a" if o == 0 else "bi1b"][:, 0:1],
                                     scale=sb["sc1a" if o == 0 else "sc1b"][:, 0:1],
                                     alpha=0.01)
            z2 = ps_fc.tile([64, 1], FP, tag="fc")
            nc.tensor.matmul(z2[:], sb["l2T"][:, 0:64], z1sb[:, 0:1], start=True, stop=False)
            nc.tensor.matmul(z2[:], sb["l2T"][:, 64:128], z1sb[:, 1:2], start=False, stop=True)
            z2sb = tp.tile([64, 1], FP, tag="z2")
            nc.scalar.activation(z2sb[:], z2[:], AF.Prelu,
                                 bias=sb["bi2"][:, 0:1], scale=sb["sc2"][:, 0:1],
                                 alpha=0.01)
            z3 = ps_fc.tile([CLS, 1], FP, tag="fc")
            nc.tensor.matmul(z3[:], sb["l3T"][:], z2sb[:], start=True, stop=True)
            z3sb = tp.tile([CLS, 1], FP, tag="z3")
            nc.scalar.activation(z3sb[:], z3[:], AF.Identity, bias=sb["bi3"][:, 0:1])
            nc.sync.dma_start(out_d[:], z3sb[:])

    nc.compile()
    return nc


def _host_prep(inputs):
    f32 = np.float32

    def bnfold(p):
        s, b, m, v = np.asarray(p, f32)
        scl = (s / np.sqrt(v + EPS)).astype(f32)
        return scl, (b - m * scl).astype(f32)

    w = {}
    for b in range(1, 5):
        C = CS[b - 1]
        wb = np.asarray(inputs[f"w{b}"], f32)
        wa, wrest = wb[:, :C], wb[:, C:]
        wd = (wrest - wa).astype(f32)
        scl, bi = bnfold(inputs[f"bn{b}"])
        w[f"waaT{b}"] = (np.concatenate([wa.T, wa.T], axis=1) * np.tile(scl, 2)[None, :]).astype(f32)
        w[f"wddT{b}"] = np.concatenate([wd.T, wd.T], axis=1).astype(f32)
        w[f"scdup{b}"] = np.tile(scl, 2)[:, None]
        w[f"bidup{b}"] = np.tile(bi, 2)[:, None]
        w.setdefault("_scl", {})[b] = (scl, bi)
    scl1, bi1 = w["_scl"][1]; scl2, bi2 = w["_scl"][2]
    scl3, bi3 = w["_scl"][3]; scl4, bi4 = w["_scl"][4]
    w["ident"] = np.eye(128, dtype=f32)
    del w["_scl"]

    w5 = np.asarray(inputs["w5"], f32)
    w["w5T12"] = w5[:, 0:128].T.copy()
    w["w5T34"] = w5[:, 128:256].T.copy()
    s5, b5 = bnfold(inputs["bn5"])
    w["sc5"], w["bi5"] = s5[:, None], b5[:, None]

    w6 = np.asarray(inputs["w6"], f32)
    w["w6Ta"] = w6[0:128, :].T.copy()
    w["w6Tb"] = w6[128:256, :].T.copy()
    s6, b6 = bnfold(inputs["bn6"])
    w["sc6a"], w["bi6a"] = s6[0:128, None], b6[0:128, None]
    w["sc6b"], w["bi6b"] = s6[128:256, None], b6[128:256, None]

    lw1 = np.asarray(inputs["lw1"], f32)
    lb1 = np.asarray(inputs["lb1"], f32)
    sl1, bb1 = bnfold(inputs["lbn1"])
    bias1 = (lb1 * sl1 + bb1).astype(f32)
    LW1 = np.concatenate([lw1[:, 0:256], lw1[:, 256:512] / 2048.0], axis=1).astype(f32)
    l1T = np.zeros((128, 1024), f32)
    for k in range(4):
        for o in range(2):
            l1T[:, (k * 2 + o) * 128:(k * 2 + o + 1) * 128] = \
                LW1[o * 128:(o + 1) * 128, k * 128:(k + 1) * 128].T
    w["l1T"] = l1T
    w["sc1a"], w["bi1a"] = sl1[0:128, None], bias1[0:128, None]
    w["sc1b"], w["bi1b"] = sl1[128:256, None], bias1[128:256, None]

    lw2 = np.asarray(inputs["lw2"], f32)
    lb2 = np.asarray(inputs["lb2"], f32)
    sl2, bb2 = bnfold(inputs["lbn2"])
    l2T = np.zeros((128, 128), f32)
    l2T[:, 0:64] = lw2[:, 0:128].T
    l2T[:, 64:128] = lw2[:, 128:256].T
    w["l2T"] = l2T
    w["sc2"] = sl2[:, None]
    w["bi2"] = (lb2 * sl2 + bb2)[:, None]

    w["l3T"] = np.asarray(inputs["lw3"], f32).T.copy()
    w["bi3"] = np.asarray(inputs["lb3"], f32)[:, None]
    return w


def kernel(**inputs):
    if "nc" not in _CACHE:
        _CACHE["nc"] = _build()
    nc = _CACHE["nc"]
    w = _host_prep(inputs)
    x = np.asarray(inputs["x"], np.float32)
    in_maps = []
    for i in range(B):
        m = dict(w)
        m["x3"] = np.concatenate([x[i], np.ones((1, N), np.float32)], axis=0)
        m["s2one"] = np.concatenate(
            [2.0 * x[i], -(x[i] * x[i]).sum(axis=0, keepdims=True)],
            axis=0).astype(np.float32)
        m["onesrow"] = np.ones((1, N), np.float32)
        in_maps.append(m)
    res = run_bass_kernel_spmd(nc, in_maps, list(range(B)))
    return np.stack([res.results[i]["out"].reshape(CLS) for i in range(B)]).astype(np.float32)



# revision 43
# speedup vs baseline: 1.0055x; 1.0055x over previous
"""DGCNN (nn_DGCNN_11106785427638) Trainium2 Bass kernel.

Pure data-parallel: B=8 samples sharded 1-per-core across 8 NeuronCores.
Per core (N=2048 points, k=20, f32 throughout):

  Per EdgeConv block b=1..4 (C=3,64,64,64):
    - knn: dist tile (128 rows, 2048) via PE matmul with the -|x_m|^2 term
      folded in as an extra contraction row: s[n,m] = 2<x_n,x_m> - |x_m|^2
      (rank-equivalent to the reference's pairwise -dist^2).
    - exact sorted top-20 via DVE max8/max_index/match_replace (3 rounds).
    - neighbor gather runs on GPSIMD ap_gather over A_b = Wa_b @ x_b
      (the 1x1 conv is linear, so conv(gather(x)) == gather(A)); the center
      term Bv_b = (Wb_b - Wa_b) @ x_b is added per-point afterwards.
    - x_{b+1} = prelu(maxpool_j(gather(A)) * s + (Bv*s + bias)) using the
      monotonicity of the (positive-scale) BN + leaky relu.
  Phase 2: y_b = prelu((A_b[idx] )*s + BvS_b) at full (64, N, 20), conv5 as
  two K=128 pair-stacked matmuls, maxpool_j, conv6, global max/mean pool,
  3-layer MLP head. Weights/BN folds are host-preprocessed.

Flat gather order within a 128-row tile: i = a*320 + j*16 + p with local
row n = 16a + p, neighbor rank j. The idx "wrap" for ap_gather is one
SBUF->DRAM bounce in (p, a, j) layout plus stride-0 broadcast reads that
replicate it per 16-partition core group (1 DMA for idxw, 2 for idxh),
splits phase-1 across all 8 Q7 cores by row-halves, and keeps pooled
outputs n-contiguous.

Scheduling: S2 = [2x; -|x|^2] for block b+1 is built incrementally per
128-col chunk as S1[b] tiles complete (block 1's S2 and [x;1] come
host-side), each block's first two dist tiles are emitted ahead of the
aa/bb prestep, and conv6 + global pools run chunked inside the phase-2
loop - together hiding most block-transition/startup/tail bubbles. The
exact top-20 (3x max8/max_index + 2x match_replace = 8 DVE passes/tile)
is the DVE-bound floor: ~1.34 ms busy of ~1.44 ms total.
"""
import sys

sys.path.insert(0, "/opt/trn_rl_repo")
import numpy as np
import concourse.bacc as bacc
import concourse.tile as tile
from concourse import mybir
from concourse.bass_utils import run_bass_kernel_spmd
import contextlib
from contextlib import ExitStack

FP = mybir.dt.float32
U32 = mybir.dt.uint32
U16 = mybir.dt.uint16
I16 = mybir.dt.int16
AF = mybir.ActivationFunctionType
ALU = mybir.AluOpType
AX = mybir.AxisListType

B, N, K, CLS = 8, 2048, 20, 40
NT = N // 128          # 16 row tiles
NEG = -3.0e38
EPS = 1e-5
CS = [3, 64, 64, 64]   # per-block input channels

_CACHE = {}


def _build():
    nc = bacc.Bacc("TRN2", target_bir_lowering=False, debug=False)

    d = {}
    def din(name, shape, dt=FP):
        d[name] = nc.dram_tensor(name, list(shape), dt, kind="ExternalInput").ap()
        return d[name]

    din("x3", (4, N))
    din("s2one", (4, N))
    din("onesrow", (1, N))
    for b in range(1, 5):
        C = CS[b - 1]
        din(f"waaT{b}", (C, 128))
        din(f"wddT{b}", (C, 128))
        din(f"scdup{b}", (128, 1))
        din(f"bidup{b}", (128, 1))
    din("ident", (128, 128))
    din("w5T12", (128, 64)); din("w5T34", (128, 64))
    din("sc5", (64, 1)); din("bi5", (64, 1))
    din("w6Ta", (64, 128)); din("w6Tb", (64, 128))
    din("sc6a", (128, 1)); din("bi6a", (128, 1))
    din("sc6b", (128, 1)); din("bi6b", (128, 1))
    din("l1T", (128, 1024))
    din("sc1a", (128, 1)); din("bi1a", (128, 1))
    din("sc1b", (128, 1)); din("bi1b", (128, 1))
    din("l2T", (128, 128)); din("sc2", (64, 1)); din("bi2", (64, 1))
    din("l3T", (64, CLS)); din("bi3", (CLS, 1))
    out_d = nc.dram_tensor("out", [CLS, 1], FP, kind="ExternalOutput").ap()

    bounce_m = {}
    for b in range(1, 5):
        for t in range(NT):
            bounce_m[(b, t)] = nc.dram_tensor(f"bncm{b}_{t}", [16, 160], U16)

    with tile.TileContext(nc) as tc, ExitStack() as ctx:
        cp = ctx.enter_context(tc.tile_pool(name="consts", bufs=1))
        sb = {}
        # block-1-critical inputs first so the first dist tile starts ASAP;
        # remaining consts (phase-2/tail weights) stream in behind them.
        early = ["x3", "s2one", "onesrow", "waaT1", "wddT1", "scdup1", "bidup1"]
        for name in early + [n for n in d if n not in early]:
            ap = d[name]
            t_ = cp.tile(list(ap.shape), ap.dtype, name=f"c_{name}", tag=f"c_{name}")
            nc.sync.dma_start(t_[:], ap[:])
            sb[name] = t_
        negones = cp.tile([64, 1], FP)
        nc.vector.memset(negones[:], -1.0)
        # dummy activation: pulls the ACT function-table load into the
        # initial const-DMA wait instead of the first dist-copy
        actwarm = cp.tile([64, 1], FP)
        nc.scalar.activation(actwarm[:], negones[:], AF.Prelu, alpha=0.2)

        persist = ctx.enter_context(tc.tile_pool(name="persist", bufs=1))
        S1 = {i: persist.tile([65, N], FP, name=f"S1_{i}", tag=f"S1_{i}") for i in (1, 2, 3)}
        pairA = {p: persist.tile([128, N], FP, name=f"pairA{p}", tag=f"pairA{p}") for p in (12, 34)}
        BvSpair = {p: persist.tile([128, N], FP, name=f"BvSpair{p}", tag=f"BvSpair{p}") for p in (12, 34)}
        idxw = {p: persist.tile([128, NT * 160], U16, name=f"idxw{p}", tag=f"idxw{p}") for p in (12, 34)}
        hmax = persist.tile([64, N], FP)
        g12pre = {0: persist.tile([128, 2560], FP, name="g12pre0", tag="g12pre0")}

        # phase-2 front stage (gather -> +BvS -> prelu); also used to pre-run
        # pair 12 of tile 0 during block 4 (its inputs are ready after block 2)
        def front_one(pair, t, G):
            nc.gpsimd.ap_gather(G[:], pairA[pair][:],
                                idxw[pair][:, t * 160:(t + 1) * 160].bitcast(I16),
                                channels=128, num_elems=N, d=1, num_idxs=2560)
            for a in range(8):
                gv = G[:, a * 320:(a + 1) * 320].rearrange(
                    "c (j p) -> c j p", j=20, p=16)
                bvv = BvSpair[pair][:, t * 128 + a * 16:t * 128 + (a + 1) * 16] \
                    .rearrange("c (u2 p) -> c u2 p", u2=1) \
                    .broadcast_to([128, 20, 16])
                nc.vector.scalar_tensor_tensor(gv, gv, 1.0, bvv,
                                               op0=ALU.mult, op1=ALU.add)
            for g in range(4):
                nc.scalar.activation(G[:, g * 640:(g + 1) * 640],
                                     G[:, g * 640:(g + 1) * 640],
                                     AF.Prelu, alpha=0.2)
            return G

        # ---------------- phase 1: blocks ----------------
        with tc.tile_pool(name="s2p", bufs=2) as s2p, \
             tc.tile_pool(name="xsqc", bufs=2) as xsqcp, \
             tc.tile_pool(name="adup", bufs=1) as adupp, \
             tc.tile_pool(name="bvs", bufs=1) as bvsp, \
             tc.tile_pool(name="dist", bufs=3) as distp, \
             tc.tile_pool(name="gph1", bufs=2) as gph1p, \
             tc.tile_pool(name="small", bufs=4) as smallp, \
             tc.tile_pool(name="ps_pre", bufs=1, space="PSUM") as ps_pre, \
             tc.tile_pool(name="ps_dist", bufs=3, space="PSUM") as ps_dist:
            S2next = None
            pending_chunk = None
            for b in range(1, 5):
                C = CS[b - 1]
                pair = 12 if b <= 2 else 34
                half = slice(0, 64) if b % 2 == 1 else slice(64, 128)
                qbase = 0 if b % 2 == 1 else 4
                # block 1 reads [x3; 1] straight from the const tile; S2_1 is
                # host-computed ("s2one"); later blocks use the incrementally
                # built S1/S2next.
                xfull = sb["x3"] if b == 1 else S1[b - 1]
                x_b = xfull[0:C, :]
                S2 = sb["s2one"] if b == 1 else S2next
                if b <= 3:
                    S2next = s2p.tile([65, N], FP, tag="S2n")

                # first dist tile ahead of the aa/bb prestep: its inputs are
                # ready before x_b's last chunk, and PE/ACT queues are
                # in-order, so this shortens every block transition.
                def emit_dist(t, mid=None):
                    lhsT = xfull[:, t * 128:(t + 1) * 128]
                    dist = distp.tile([128, N], FP, name=f"dist_{b}_{t}", tag="dist")
                    for ch in range(2):
                        cs = slice(ch * 1024, (ch + 1) * 1024)
                        dps = ps_dist.tile([128, 1024], FP, name=f"dps_{b}_{t}_{ch}", tag="dch")
                        if ch == 1 and mid is not None:
                            # transition tile: finer pieces so only the last
                            # 128 cols depend on the previous block's final
                            # S2 chunk
                            for lo, hi in ((0, 512), (512, 896), (896, 1024)):
                                nc.tensor.matmul(dps[:, lo:hi], lhsT,
                                                 S2[0:C + 1, 1024 + lo:1024 + hi],
                                                 start=True, stop=True)
                                with tc.high_priority():
                                    nc.scalar.copy(dist[:, 1024 + lo:1024 + hi],
                                                   dps[:, lo:hi])
                            continue
                        for u in range(2):
                            nc.tensor.matmul(dps[:, u * 512:(u + 1) * 512], lhsT,
                                             S2[0:C + 1, ch * 1024 + u * 512:ch * 1024 + (u + 1) * 512],
                                             start=True, stop=True)
                        nc.scalar.copy(dist[:, cs], dps[:])
                        if ch == 0 and mid is not None:
                            with tc.high_priority():
                                mid()
                    return dist
                # the previous block's tile-15 S2 chunk is emitted between
                # dist-t0's column halves: half 0 doesn't depend on it, so PE
                # starts early instead of head-of-line blocking on it.
                dist0 = emit_dist(0, mid=pending_chunk)
                pending_chunk = None
                dist1 = emit_dist(1)

                if b <= 3:
                    Adup = adupp.tile([128, N], FP, tag="adup")
                    BvSd = bvsp.tile([128, N], FP, tag="bvs")
                for hh in range(2):
                    hs = slice(hh * 1024, (hh + 1) * 1024)
                    aa = ps_pre.tile([128, 1024], FP, tag="pre")
                    for ch in range(2):
                        nc.tensor.matmul(aa[:, ch * 512:(ch + 1) * 512], sb[f"waaT{b}"][:],
                                         x_b[:, hh * 1024 + ch * 512:hh * 1024 + (ch + 1) * 512],
                                         start=True, stop=True)
                    nc.scalar.copy(pairA[pair][half, hs], aa[half, :])
                    if b <= 3:
                        nc.scalar.copy(Adup[:, hs], aa[:])

                    bb = ps_pre.tile([128, 1024], FP, tag="pre")
                    for ch in range(2):
                        nc.tensor.matmul(bb[:, ch * 512:(ch + 1) * 512], sb[f"wddT{b}"][:],
                                         x_b[:, hh * 1024 + ch * 512:hh * 1024 + (ch + 1) * 512],
                                         start=True, stop=True)
                    nc.scalar.activation(BvSpair[pair][half, hs], bb[half, :], AF.Identity,
                                         bias=sb[f"bidup{b}"][half, 0:1],
                                         scale=sb[f"scdup{b}"][half, 0:1])
                    if b <= 3:
                        nc.scalar.activation(BvSd[:, hs], bb[:], AF.Identity,
                                             bias=sb[f"bidup{b}"][:, 0:1],
                                             scale=sb[f"scdup{b}"][:, 0:1])
                if b <= 3:
                    nc.scalar.copy(S1[b][64:65, :], sb["onesrow"][:])
                if b == 4:
                    # pair-12 front work of phase-2 tile 0 (inputs ready once
                    # block 2 is done): fills the block-3/4 transition bubble
                    # on DVE/ACT/Pool
                    front_one(12, 0, g12pre[0])

                for t in range(NT):
                    dist = dist0 if t == 0 else (dist1 if t == 1 else emit_dist(t))

                    vals = smallp.tile([128, 24], FP, tag="vals")
                    idx16 = smallp.tile([128, 24], U16, tag="idx16")
                    for r in range(3):
                        nc.vector.max(vals[:, r * 8:(r + 1) * 8], dist[:])
                        nc.vector.max_index(idx16[:, r * 8:(r + 1) * 8],
                                            vals[:, r * 8:(r + 1) * 8], dist[:])
                        if r < 2:
                            nc.vector.match_replace(dist[:], vals[:, r * 8:(r + 1) * 8],
                                                    dist[:], NEG)

                    # the last tile's post-topk chain is the block-transition
                    # critical path: tell the scheduler to prefer it over
                    # competing ready work on the same engines
                    last = t == NT - 1
                    hp = tc.high_priority() if t >= NT - 2 else contextlib.nullcontext()
                    with hp:
                        # idx wrap via DRAM bounce in (p, a, j) layout; read
                        # back with a stride-0 4x quadrant broadcast
                        m_ap = bounce_m[(b, t)].ap()
                        nc.sync.dma_start(m_ap.rearrange("p (a j) -> a p j", a=8, j=20),
                                          idx16[:, 0:20])
                        nc.sync.dma_start(
                            idxw[pair][qbase * 16:(qbase + 4) * 16, t * 160:(t + 1) * 160],
                            m_ap.unsqueeze(0).broadcast_to([4, 16, 160]))

                        if b <= 3:
                            idxh = smallp.tile([128, 80], U16, tag="idxh")
                            nc.sync.dma_start(
                                idxh[0:64, :],
                                m_ap[:, 0:80].unsqueeze(0).broadcast_to([4, 16, 80]))
                            nc.sync.dma_start(
                                idxh[64:128, :],
                                m_ap[:, 80:160].unsqueeze(0).broadcast_to([4, 16, 80]))
                            G = gph1p.tile([128, 1280], FP, tag="g1")
                            nc.gpsimd.ap_gather(G[:], Adup[:], idxh[:].bitcast(I16),
                                                channels=128, num_elems=N, d=1, num_idxs=1280)
                            Rt = smallp.tile([128, 64], FP, tag="rt")
                            nc.vector.tensor_reduce(
                                Rt[:], G[:].rearrange("c (a j p) -> c a p j", a=4, j=20, p=16),
                                AX.X, ALU.max)
                            t1 = smallp.tile([128, 64], FP, tag="t1")
                            # t1 on Pool in steady state; on DVE for the last
                            # tile (DVE idles in the drain; skips a queue hop)
                            t1eng = nc.vector if last else nc.gpsimd
                            t1eng.tensor_tensor(
                                t1[0:64, :], Rt[0:64, :],
                                BvSd[0:64, t * 128:t * 128 + 64], ALU.add)
                            t1eng.tensor_tensor(
                                t1[64:128, :], Rt[64:128, :],
                                BvSd[64:128, t * 128 + 64:(t + 1) * 128], ALU.add)
                            t2 = smallp.tile([128, 64], FP, tag="t2")
                            nc.scalar.activation(t2[:], t1[:], AF.Prelu, alpha=0.2)
                            nc.scalar.copy(S1[b][0:64, t * 128:t * 128 + 64], t2[0:64, :])
                            nc.sync.dma_start(S1[b][0:64, t * 128 + 64:(t + 1) * 128],
                                              t2[64:128, :])

                            # incremental S2 build for block b+1 over these cols
                            def chunk_build(t=t, S1b=S1[b], S2n=S2next):
                                tcols = slice(t * 128, (t + 1) * 128)
                                xsqc = xsqcp.tile([64, 128], FP, tag="xsqc")
                                nc.scalar.activation(xsqc[:], S1b[0:64, tcols], AF.Square)
                                nxxc = ps_pre.tile([1, 128], FP, tag="pre")
                                nc.tensor.matmul(nxxc[:], negones[:], xsqc[:],
                                                 start=True, stop=True)
                                nc.scalar.mul(S2n[0:64, tcols], S1b[0:64, tcols], 2.0)
                                nc.scalar.copy(S2n[64:65, tcols], nxxc[:])
                            if not last:
                                chunk_build()
                            else:
                                pending_chunk = chunk_build

        # ---------------- phase 2: y + conv5 + pool ----------------
        # z = diag(s) @ G + I @ BvS (PE identity-adds; BvS broadcast over j
        # as a 0-stride rhs); y = prelu(z) with y12 on ACT, y34 on DVE via
        # max(0.2*v, v) to balance engine load.
        with tc.tile_pool(name="g2", bufs=3) as g2p, \
             tc.tile_pool(name="hsb", bufs=2) as hsbp, \
             tc.tile_pool(name="tail", bufs=1) as tp, \
             tc.tile_pool(name="ps_h", bufs=3, space="PSUM") as ps_h, \
             tc.tile_pool(name="ps_t6", bufs=2, space="PSUM") as ps_t6, \
             tc.tile_pool(name="ps_fc", bufs=2, space="PSUM") as ps_fc:
            # conv6 + global max/sum run chunked, interleaved into the
            # phase-2 loop as hmax columns complete
            gmall = tp.tile([128, 8], FP, tag="gmall")
            gsall = tp.tile([128, 8], FP, tag="gsall")
            W6 = (("w6Ta", "sc6a", "bi6a"), ("w6Tb", "sc6b", "bi6b"))

            def conv6_chunk(ch):
                cs = slice(ch * 512, (ch + 1) * 512)
                for wi, (wname, scn, bin_) in enumerate(W6):
                    z6 = ps_t6.tile([128, 512], FP, name=f"z6_{ch}_{wi}", tag="z6")
                    nc.tensor.matmul(z6[:], sb[wname][:], hmax[:, cs],
                                     start=True, stop=True)
                    h6 = tp.tile([128, 512], FP, name=f"h6_{ch}_{wi}",
                                 tag="h6c", bufs=2)
                    nc.scalar.activation(h6[:], z6[:], AF.Prelu,
                                         bias=sb[bin_][:, 0:1],
                                         scale=sb[scn][:, 0:1], alpha=0.2)
                    col = slice(wi * 4 + ch, wi * 4 + ch + 1)
                    nc.vector.tensor_reduce(gmall[:, col], h6[:], AX.X, ALU.max)
                    nc.vector.tensor_reduce(gsall[:, col], h6[:], AX.X, ALU.add)
            def phase2_front(t):
                ys = {}
                for pair in (12, 34):
                    if pair == 12 and t == 0:
                        ys[12] = g12pre[0]  # pre-computed during block 4
                        continue
                    G = g2p.tile([128, 2560], FP, name=f"g{pair}_{t}", tag=f"g{pair}")
                    ys[pair] = front_one(pair, t, G)
                return ys

            def phase2_back(t, ys):
                h_sb = hsbp.tile([64, 2560], FP, name=f"hsb_{t}", tag="hsb")
                for ch in range(5):
                    cs = slice(ch * 512, (ch + 1) * 512)
                    hps = ps_h.tile([64, 512], FP, name=f"hps_{t}_{ch}", tag="h")
                    nc.tensor.matmul(hps[:], sb["w5T12"][:], ys[12][:, cs],
                                     start=True, stop=False)
                    nc.tensor.matmul(hps[:], sb["w5T34"][:], ys[34][:, cs],
                                     start=False, stop=True)
                    nc.scalar.activation(h_sb[:, cs], hps[:], AF.Prelu,
                                         bias=sb["bi5"][:, 0:1], scale=sb["sc5"][:, 0:1],
                                         alpha=0.2)
                nc.vector.tensor_reduce(
                    hmax[:, t * 128:(t + 1) * 128],
                    h_sb[:].rearrange("c (a j p) -> c a p j", a=8, j=20, p=16),
                    AX.X, ALU.max)

            ys_prev = None
            for t in range(NT + 1):
                ys_cur = phase2_front(t) if t < NT else None
                if ys_prev is not None:
                    phase2_back(t - 1, ys_prev)
                    if (t - 1) % 4 == 3:
                        ch = (t - 1) // 4
                        if ch == 3:
                            with tc.high_priority():
                                conv6_chunk(ch)
                        else:
                            conv6_chunk(ch)
                ys_prev = ys_cur

            # ---------------- tail: final pools + MLP ----------------
            tailhp = ctx.enter_context(tc.high_priority())
            gpieces = []
            for wi in range(2):
                gm = tp.tile([128, 1], FP, name=f"gm{wi}", tag=f"gm{wi}")
                nc.vector.tensor_reduce(gm[:], gmall[:, wi * 4:(wi + 1) * 4],
                                        AX.X, ALU.max)
                gs = tp.tile([128, 1], FP, name=f"gs{wi}", tag=f"gs{wi}")
                nc.vector.tensor_reduce(gs[:], gsall[:, wi * 4:(wi + 1) * 4],
                                        AX.X, ALU.add)
                gpieces.append((gm, gs))
            gchunks = [gpieces[0][0], gpieces[1][0], gpieces[0][1], gpieces[1][1]]

            z1sb = tp.tile([128, 2], FP, tag="z1")
            for o in range(2):
                z1 = ps_fc.tile([128, 1], FP, tag="fc")
                for k in range(4):
                    nc.tensor.matmul(z1[:], sb["l1T"][:, (k * 2 + o) * 128:(k * 2 + o + 1) * 128],
                                     gchunks[k][:], start=(k == 0), stop=(k == 3))
                nc.scalar.activation(z1sb[:, o:o + 1], z1[:], AF.Prelu,
                                     bias=sb["bi1a" if o == 0 else "bi1b"][:, 0:1],
                                     scale=sb["sc1a" if o == 0 else "sc1b"][:, 0:1],
                                     alpha=0.01)
            z2 = ps_fc.tile([64, 1], FP, tag="fc")
            nc.tensor.matmul(z2[:], sb["l2T"][:, 0:64], z1sb[:, 0:1], start=True, stop=False)
            nc.tensor.matmul(z2[:], sb["l2T"][:, 64:128], z1sb[:, 1:2], start=False, stop=True)
            z2sb = tp.tile([64, 1], FP, tag="z2")
            nc.scalar.activation(z2sb[:], z2[:], AF.Prelu,
                                 bias=sb["bi2"][:, 0:1], scale=sb["sc2"][:, 0:1],
                                 alpha=0.01)
            z3 = ps_fc.tile([CLS, 1], FP, tag="fc")
            nc.tensor.matmul(z3[:], sb["l3T"][:], z2sb[:], start=True, stop=True)
            z3sb = tp.tile([CLS, 1], FP, tag="z3")
            nc.scalar.activation(z3sb[:], z3[:], AF.Identity, bias=sb["bi3"][:, 0:1])
            nc.sync.dma_start(out_d[:], z3sb[:])

    nc.compile()
    return nc


def _host_prep(inputs):
    f32 = np.float32

    def bnfold(p):
        s, b, m, v = np.asarray(p, f32)
        scl = (s / np.sqrt(v + EPS)).astype(f32)
        return scl, (b - m * scl).astype(f32)

    w = {}
    for b in range(1, 5):
        C = CS[b - 1]
        wb = np.asarray(inputs[f"w{b}"], f32)
        wa, wrest = wb[:, :C], wb[:, C:]
        wd = (wrest - wa).astype(f32)
        scl, bi = bnfold(inputs[f"bn{b}"])
        w[f"waaT{b}"] = (np.concatenate([wa.T, wa.T], axis=1) * np.tile(scl, 2)[None, :]).astype(f32)
        w[f"wddT{b}"] = np.concatenate([wd.T, wd.T], axis=1).astype(f32)
        w[f"scdup{b}"] = np.tile(scl, 2)[:, None]
        w[f"bidup{b}"] = np.tile(bi, 2)[:, None]
        w.setdefault("_scl", {})[b] = (scl, bi)
    scl1, bi1 = w["_scl"][1]; scl2, bi2 = w["_scl"][2]
    scl3, bi3 = w["_scl"][3]; scl4, bi4 = w["_scl"][4]
    w["ident"] = np.eye(128, dtype=f32)
    del w["_scl"]

    w5 = np.asarray(inputs["w5"], f32)
    w["w5T12"] = w5[:, 0:128].T.copy()
    w["w5T34"] = w5[:, 128:256].T.copy()
    s5, b5 = bnfold(inputs["bn5"])
    w["sc5"], w["bi5"] = s5[:, None], b5[:, None]

    w6 = np.asarray(inputs["w6"], f32)
    w["w6Ta"] = w6[0:128, :].T.copy()
    w["w6Tb"] = w6[128:256, :].T.copy()
    s6, b6 = bnfold(inputs["bn6"])
    w["sc6a"], w["bi6a"] = s6[0:128, None], b6[0:128, None]
    w["sc6b"], w["bi6b"] = s6[128:256, None], b6[128:256, None]

    lw1 = np.asarray(inputs["lw1"], f32)
    lb1 = np.asarray(inputs["lb1"], f32)
    sl1, bb1 = bnfold(inputs["lbn1"])
    bias1 = (lb1 * sl1 + bb1).astype(f32)
    LW1 = np.concatenate([lw1[:, 0:256], lw1[:, 256:512] / 2048.0], axis=1).astype(f32)
    l1T = np.zeros((128, 1024), f32)
    for k in range(4):
        for o in range(2):
            l1T[:, (k * 2 + o) * 128:(k * 2 + o + 1) * 128] = \
                LW1[o * 128:(o + 1) * 128, k * 128:(k + 1) * 128].T
    w["l1T"] = l1T
    w["sc1a"], w["bi1a"] = sl1[0:128, None], bias1[0:128, None]
    w["sc1b"], w["bi1b"] = sl1[128:256, None], bias1[128:256, None]

    lw2 = np.asarray(inputs["lw2"], f32)
    lb2 = np.asarray(inputs["lb2"], f32)
    sl2, bb2 = bnfold(inputs["lbn2"])
    l2T = np.zeros((128, 128), f32)
    l2T[:, 0:64] = lw2[:, 0:128].T
    l2T[:, 64:128] = lw2[:, 128:256].T
    w["l2T"] = l2T
    w["sc2"] = sl2[:, None]
    w["bi2"] = (lb2 * sl2 + bb2)[:, None]

    w["l3T"] = np.asarray(inputs["lw3"], f32).T.copy()
    w["bi3"] = np.asarray(inputs["lb3"], f32)[:, None]
    return w


def kernel(**inputs):
    if "nc" not in _CACHE:
        _CACHE["nc"] = _build()
    nc = _CACHE["nc"]
    w = _host_prep(inputs)
    x = np.asarray(inputs["x"], np.float32)
    in_maps = []
    for i in range(B):
        m = dict(w)
        m["x3"] = np.concatenate([x[i], np.ones((1, N), np.float32)], axis=0)
        m["s2one"] = np.concatenate(
            [2.0 * x[i], -(x[i] * x[i]).sum(axis=0, keepdims=True)],
            axis=0).astype(np.float32)
        m["onesrow"] = np.ones((1, N), np.float32)
        in_maps.append(m)
    res = run_bass_kernel_spmd(nc, in_maps, list(range(B)))
    return np.stack([res.results[i]["out"].reshape(CLS) for i in range(B)]).astype(np.float32)

